# revision 15
# baseline (speedup 1.0000x reference)
"""Trainium2 Bass kernel for the box-smoothed Charbonnier loss.

reference:  diff = conv7x7_box(sum_ch(x - y)) / 49 ;  loss = mean(sqrt(diff^2 + 1e-6))

Strategy (pure data parallel, 2 images per core on 8 cores):
  - Row-interleaved ("p-major") SBUF layout: partition p holds rows
    4p..4p+3, so DRAM runs are 8KB-contiguous and each tensor-image loads
    as ONE 3MB 3-dim DMA (descriptor-gen is the HWDGE serializer).
    x loads ride the SP HWDGE ring, y loads the gpsimd SWDGE ring.
  - DVE computes s = sum_ch(x - y) per image.
  - 7-wide box conv in each direction is a banded-matrix matmul on the PE
    in float32r (1 cycle/col vs 4 for fp32 at N=512). Band rides as the
    moving operand, image data as the stationary one, fusing conv+transpose:
        stage1[m, n] = sum_r s[r, 128*cb + m] * band(r, n)   (vertical, transposed out)
        stage2[m, n] = sum_w t[w, 128*hb + m] * band(w, n)   (horizontal, back to [h, w])
    Stage1 uses the p-major band (rows 4p+slot), stage2 the chunk-major one.
  - Charbonnier on ACT: Square (PSUM->SBUF), Sqrt(x + eps) with accum_out
    giving per-partition sums; cross-partition total via a ones-matmul.
  - Host sums the 8 per-core partials and divides by the element count.
"""

import numpy as np

import concourse.bass as bass
import concourse.bacc as bacc
import concourse.mybir as mybir
import concourse.tile as tile
from concourse.bass_interp import get_hw_module
from concourse.bass_utils import run_bass_kernel_spmd

N_CORES = 8
B_TOTAL = 16
B_PER_CORE = B_TOTAL // N_CORES
CH = 3
H = W = 512
P = 128
NCHUNK = H // P  # 4
EPS = 1e-6
F32 = mybir.dt.float32
F32R = mybir.dt.float32r
AF = mybir.ActivationFunctionType


def make_band() -> np.ndarray:
    """[128, 8, 512]; slots 0..3: row = 4p+slot (stage1, p-major),
    slots 4..7: row = 128*(slot-4)+p (stage2, chunk-major); value 1/7 in band."""
    band = np.zeros((P, 2 * NCHUNK, W), dtype=np.float32)
    p = np.arange(P)[:, None, None]
    slot = np.arange(NCHUNK)[None, :, None]
    n = np.arange(W)[None, None, :]
    seventh = np.float32(1.0) / np.float32(7.0)
    band[:, :NCHUNK][np.abs(4 * p + slot - n) <= 3] = seventh
    band[:, NCHUNK:][np.abs(P * slot + p - n) <= 3] = seventh
    return band


def build_program() -> tuple[bacc.Bacc, str, str, str, str]:
    nc = bacc.Bacc("TRN2", target_bir_lowering=False, debug=False, num_devices=N_CORES)

    x = nc.dram_tensor("x", [B_PER_CORE, CH, H, W], F32, kind="ExternalInput")
    y = nc.dram_tensor("y", [B_PER_CORE, CH, H, W], F32, kind="ExternalInput")
    band = nc.dram_tensor("band", [P, 2 * NCHUNK, W], F32, kind="ExternalInput")
    out = nc.dram_tensor("out", [1, 1], F32, kind="ExternalOutput")

    add = mybir.AluOpType.add

    with tile.TileContext(nc) as tc:
        with (
            tc.tile_pool(name="const", bufs=1) as cpool,
            tc.tile_pool(name="xy", bufs=1) as xypool,
            tc.tile_pool(name="data", bufs=2) as dpool,
            tc.tile_pool(name="small", bufs=2) as spool,
            tc.tile_pool(name="psum", bufs=2, space="PSUM") as ppool,
            tc.tile_pool(name="psum1", bufs=1, space="PSUM") as ppool1,
        ):
            band_f = cpool.tile([P, 2 * NCHUNK, W], F32)
            nc.sync.dma_start(band_f[:], band.ap()[:])
            # fp32r consumers need a rounding producer; DVE copy rounds on cast
            band_t = cpool.tile([P, 2 * NCHUNK, W], F32R)
            nc.vector.tensor_copy(band_t[:], band_f[:])

            ones = cpool.tile([P, 1], F32)
            nc.gpsimd.memset(ones[:], 1.0)
            epsb = cpool.tile([P, 1], F32)
            nc.gpsimd.memset(epsb[:], float(EPS))

            acc = cpool.tile([P, B_PER_CORE * NCHUNK], F32)

            # prefetch whole-image loads: one 3MB DMA per (b, tensor).
            # p-major rows: partition p <- rows 4p..4p+3  => 8KB DRAM runs.
            xt, yt = [], []
            for b in range(B_PER_CORE):
                xb = xypool.tile([P, CH, NCHUNK, W], F32, tag=f"x{b}")
                yb = xypool.tile([P, CH, NCHUNK, W], F32, tag=f"y{b}")
                nc.sync.dma_start(
                    xb[:],
                    x.ap()[b].rearrange("ch (p c) w -> p ch (c w)", c=NCHUNK),
                )
                nc.gpsimd.dma_start(
                    yb[:],
                    y.ap()[b].rearrange("ch (p c) w -> p ch (c w)", c=NCHUNK),
                )
                xt.append(xb)
                yt.append(yb)

            for b in range(B_PER_CORE):
                xb, yb = xt[b], yt[b]
                # s = sum_ch (x - y)
                d = xypool.tile([P, CH, NCHUNK, W], F32, tag="d")
                s = dpool.tile([P, NCHUNK, W], F32R, tag="s")
                for ch in range(CH):
                    nc.vector.tensor_sub(d[:, ch, :, :], xb[:, ch, :, :], yb[:, ch, :, :])
                nc.vector.tensor_add(s[:], d[:, 0, :, :], d[:, 1, :, :])
                nc.vector.tensor_add(s[:], s[:], d[:, 2, :, :])

                # stage 1: vertical conv + transpose (p-major band slots 0..3)
                t = dpool.tile([P, NCHUNK, W], F32R, tag="t")
                for cb in range(NCHUNK):
                    ps1 = ppool.tile([P, W], F32, tag="ps1")
                    for c in range(NCHUNK):
                        nc.tensor.matmul(
                            ps1[:],
                            s[:, c, cb * P:(cb + 1) * P],
                            band_t[:, c, :],
                            start=(c == 0),
                            stop=(c == NCHUNK - 1),
                        )
                    nc.scalar.copy(t[:, cb, :], ps1[:])

                # stage 2: horizontal conv + transpose back (chunk-major slots 4..7)
                for hb in range(NCHUNK):
                    ps2 = ppool.tile([P, W], F32, tag="ps2")
                    for cb in range(NCHUNK):
                        nc.tensor.matmul(
                            ps2[:],
                            t[:, cb, hb * P:(hb + 1) * P],
                            band_t[:, NCHUNK + cb, :],
                            start=(cb == 0),
                            stop=(cb == NCHUNK - 1),
                        )
                    sq = spool.tile([P, W], F32, tag="sq")
                    nc.scalar.activation(sq[:], ps2[:], AF.Square)
                    u = spool.tile([P, W], F32, tag="u")
                    col = b * NCHUNK + hb
                    nc.scalar.activation(
                        u[:], sq[:], AF.Sqrt, bias=epsb[:],
                        accum_out=acc[:, col:col + 1],
                    )

            # total = sum over partitions of (sum over the 8 accum columns)
            red = cpool.tile([P, 1], F32)
            nc.vector.tensor_reduce(
                red[:], acc[:], axis=mybir.AxisListType.X, op=add
            )
            ps3 = ppool1.tile([1, 1], F32, tag="ps3")
            nc.tensor.matmul(ps3[:], red[:], ones[:], start=True, stop=True)
            res = cpool.tile([1, 1], F32)
            nc.scalar.copy(res[:], ps3[:])
            nc.sync.dma_start(out.ap()[:], res[:])

    nc.compile()
    nc.m = get_hw_module(nc.m)
    return nc, x.name, y.name, band.name, out.name


_CACHE = {}


def _get_program():
    if "prog" not in _CACHE:
        _CACHE["prog"] = build_program()
    return _CACHE["prog"]


def run_sharded(x: np.ndarray, y: np.ndarray, trace: bool = False):
    """Run the SPMD kernel; returns (per-core sums list, BassKernelResults)."""
    nc, xname, yname, bandname, outname = _get_program()
    band = make_band()
    x = np.ascontiguousarray(np.asarray(x, dtype=np.float32))
    y = np.ascontiguousarray(np.asarray(y, dtype=np.float32))
    in_maps = []
    for k in range(N_CORES):
        sl = slice(k * B_PER_CORE, (k + 1) * B_PER_CORE)
        in_maps.append({
            xname: x[sl],
            yname: y[sl],
            bandname: band,
        })
    res = run_bass_kernel_spmd(
        nc, in_maps, core_ids=list(range(N_CORES)), trace=trace
    )
    sums = [float(res.results[k][outname][0, 0]) for k in range(N_CORES)]
    return sums, res


def kernel(x: np.ndarray, y: np.ndarray) -> np.ndarray:
    sums, _ = run_sharded(x, y)
    total = float(np.sum(np.asarray(sums, dtype=np.float64)))
    return np.float32(total / (B_TOTAL * H * W))


# revision 18
# speedup vs baseline: 1.0894x; 1.0894x over previous
"""Trainium2 Bass kernel for the box-smoothed Charbonnier loss.

reference:  diff = conv7x7_box(sum_ch(x - y)) / 49 ;  loss = mean(sqrt(diff^2 + 1e-6))

Strategy (pure data parallel, 2 images per core on 8 cores):
  - Row-interleaved ("p-major") SBUF layout: partition p holds rows
    4p..4p+3, so DRAM runs are 8KB-contiguous and each tensor-image loads
    as ONE 3MB 3-dim DMA (descriptor-gen is the HWDGE serializer).
    x loads ride the SP HWDGE ring, y loads the gpsimd SWDGE ring.
  - DVE computes s = sum_ch(x - y) per image.
  - 7-wide box conv in each direction is a banded-matrix matmul on the PE
    in float32r (1 cycle/col vs 4 for fp32 at N=512). Band rides as the
    moving operand, image data as the stationary one, fusing conv+transpose:
        stage1[m, n] = sum_r s[r, 128*cb + m] * band(r, n)   (vertical, transposed out)
        stage2[m, n] = sum_w t[w, 128*hb + m] * band(w, n)   (horizontal, back to [h, w])
    Stage1 uses the p-major band (rows 4p+slot), stage2 the chunk-major one.
  - Charbonnier on ACT: Square (PSUM->SBUF), Sqrt(x + eps) with accum_out
    giving per-partition sums; cross-partition total via a ones-matmul.
  - Host sums the 8 per-core partials and divides by the element count.
"""

import numpy as np

import concourse.bass as bass
import concourse.bacc as bacc
import concourse.mybir as mybir
import concourse.tile as tile
from concourse.bass_interp import get_hw_module
from concourse.bass_utils import run_bass_kernel_spmd

N_CORES = 8
B_TOTAL = 16
B_PER_CORE = B_TOTAL // N_CORES
CH = 3
H = W = 512
P = 128
NCHUNK = H // P  # 4
EPS = 1e-6
F32 = mybir.dt.float32
F32R = mybir.dt.float32r
AF = mybir.ActivationFunctionType


def make_band() -> np.ndarray:
    """[128, 8, 512]; slots 0..3: row = 4p+slot (stage1, p-major),
    slots 4..7: row = 128*(slot-4)+p (stage2, chunk-major); value 1/7 in band."""
    band = np.zeros((P, 2 * NCHUNK, W), dtype=np.float32)
    p = np.arange(P)[:, None, None]
    slot = np.arange(NCHUNK)[None, :, None]
    n = np.arange(W)[None, None, :]
    seventh = np.float32(1.0) / np.float32(7.0)
    band[:, :NCHUNK][np.abs(4 * p + slot - n) <= 3] = seventh
    band[:, NCHUNK:][np.abs(P * slot + p - n) <= 3] = seventh
    return band


def build_program() -> tuple[bacc.Bacc, str, str, str, str]:
    nc = bacc.Bacc("TRN2", target_bir_lowering=False, debug=False, num_devices=N_CORES)

    x = nc.dram_tensor("x", [B_PER_CORE, CH, H, W], F32, kind="ExternalInput")
    y = nc.dram_tensor("y", [B_PER_CORE, CH, H, W], F32, kind="ExternalInput")
    band = nc.dram_tensor("band", [P, 2 * NCHUNK, W], F32R, kind="ExternalInput")
    out = nc.dram_tensor("out", [1, 1], F32, kind="ExternalOutput")

    add = mybir.AluOpType.add

    with tile.TileContext(nc) as tc:
        with (
            tc.tile_pool(name="const", bufs=1) as cpool,
            tc.tile_pool(name="xy", bufs=1) as xypool,
            tc.tile_pool(name="data", bufs=2) as dpool,
            tc.tile_pool(name="small", bufs=2) as spool,
            tc.tile_pool(name="psum", bufs=2, space="PSUM") as ppool,
            tc.tile_pool(name="psum1", bufs=1, space="PSUM") as ppool1,
        ):
            band_t = cpool.tile([P, 2 * NCHUNK, W], F32R)
            nc.sync.dma_start(band_t[:], band.ap()[:])

            ones = cpool.tile([P, 1], F32)
            nc.gpsimd.memset(ones[:], 1.0)
            epsb = cpool.tile([P, 1], F32)
            nc.gpsimd.memset(epsb[:], float(EPS))

            acc = cpool.tile([P, B_PER_CORE * NCHUNK], F32)

            # prefetch whole-image loads: one 3MB DMA per (b, tensor).
            # p-major rows: partition p <- rows 4p..4p+3  => 8KB DRAM runs.
            xt, yt = [], []
            for b in range(B_PER_CORE):
                xb = xypool.tile([P, CH, NCHUNK, W], F32, tag=f"x{b}")
                yb = xypool.tile([P, CH, NCHUNK, W], F32, tag=f"y{b}")
                nc.sync.dma_start(
                    xb[:],
                    x.ap()[b].rearrange("ch (p c) w -> p ch (c w)", c=NCHUNK),
                )
                nc.scalar.dma_start(
                    yb[:],
                    y.ap()[b].rearrange("ch (p c) w -> p ch (c w)", c=NCHUNK),
                )
                xt.append(xb)
                yt.append(yb)

            for b in range(B_PER_CORE):
                xb, yb = xt[b], yt[b]
                # s = sum_ch (x - y)
                d = xypool.tile([P, CH, NCHUNK, W], F32, tag="d")
                s = dpool.tile([P, NCHUNK, W], F32R, tag="s")
                for ch in range(CH):
                    nc.vector.tensor_sub(d[:, ch, :, :], xb[:, ch, :, :], yb[:, ch, :, :])
                nc.vector.tensor_add(s[:], d[:, 0, :, :], d[:, 1, :, :])
                nc.vector.tensor_add(s[:], s[:], d[:, 2, :, :])

                # stage 1: vertical conv + transpose (p-major band slots 0..3)
                t = dpool.tile([P, NCHUNK, W], F32R, tag="t")
                for cb in range(NCHUNK):
                    ps1 = ppool.tile([P, W], F32, tag="ps1")
                    for c in range(NCHUNK):
                        nc.tensor.matmul(
                            ps1[:],
                            s[:, c, cb * P:(cb + 1) * P],
                            band_t[:, c, :],
                            start=(c == 0),
                            stop=(c == NCHUNK - 1),
                        )
                    nc.scalar.copy(t[:, cb, :], ps1[:])

                # stage 2: horizontal conv + transpose back (chunk-major slots 4..7)
                for hb in range(NCHUNK):
                    ps2 = ppool.tile([P, W], F32, tag="ps2")
                    for cb in range(NCHUNK):
                        nc.tensor.matmul(
                            ps2[:],
                            t[:, cb, hb * P:(hb + 1) * P],
                            band_t[:, NCHUNK + cb, :],
                            start=(cb == 0),
                            stop=(cb == NCHUNK - 1),
                        )
                    sq = spool.tile([P, W], F32, tag="sq")
                    nc.scalar.activation(sq[:], ps2[:], AF.Square)
                    u = spool.tile([P, W], F32, tag="u")
                    col = b * NCHUNK + hb
                    nc.scalar.activation(
                        u[:], sq[:], AF.Sqrt, bias=epsb[:],
                        accum_out=acc[:, col:col + 1],
                    )

            # total = sum over partitions of (sum over the 8 accum columns)
            red = cpool.tile([P, 1], F32)
            nc.vector.tensor_reduce(
                red[:], acc[:], axis=mybir.AxisListType.X, op=add
            )
            ps3 = ppool1.tile([1, 1], F32, tag="ps3")
            nc.tensor.matmul(ps3[:], red[:], ones[:], start=True, stop=True)
            res = cpool.tile([1, 1], F32)
            nc.scalar.copy(res[:], ps3[:])
            nc.sync.dma_start(out.ap()[:], res[:])

    nc.compile()
    nc.m = get_hw_module(nc.m)
    return nc, x.name, y.name, band.name, out.name


_CACHE = {}


def _get_program():
    if "prog" not in _CACHE:
        _CACHE["prog"] = build_program()
    return _CACHE["prog"]


def run_sharded(x: np.ndarray, y: np.ndarray, trace: bool = False):
    """Run the SPMD kernel; returns (per-core sums list, BassKernelResults)."""
    nc, xname, yname, bandname, outname = _get_program()
    band = make_band()
    x = np.ascontiguousarray(np.asarray(x, dtype=np.float32))
    y = np.ascontiguousarray(np.asarray(y, dtype=np.float32))
    in_maps = []
    for k in range(N_CORES):
        sl = slice(k * B_PER_CORE, (k + 1) * B_PER_CORE)
        in_maps.append({
            xname: x[sl],
            yname: y[sl],
            bandname: band,
        })
    res = run_bass_kernel_spmd(
        nc, in_maps, core_ids=list(range(N_CORES)), trace=trace
    )
    sums = [float(res.results[k][outname][0, 0]) for k in range(N_CORES)]
    return sums, res


def kernel(x: np.ndarray, y: np.ndarray) -> np.ndarray:
    sums, _ = run_sharded(x, y)
    total = float(np.sum(np.asarray(sums, dtype=np.float64)))
    return np.float32(total / (B_TOTAL * H * W))


# revision 19
# speedup vs baseline: 1.2202x; 1.1201x over previous
"""Trainium2 Bass kernel for the box-smoothed Charbonnier loss.

reference:  diff = conv7x7_box(sum_ch(x - y)) / 49 ;  loss = mean(sqrt(diff^2 + 1e-6))

Strategy (pure data parallel, 2 images per core on 8 cores):
  - Row-interleaved ("p-major") SBUF layout: partition p holds rows
    4p..4p+3, so DRAM runs are 8KB-contiguous. Loads are 1MB per-channel
    pieces, paired across the two HWDGE rings (x on SP, y on ACT) so the
    DVE difference/channel-sum chain streams behind the DMAs.
  - 7-wide box conv in each direction is a banded-matrix matmul on the PE
    in float32r (1 cycle/col vs 4 for fp32 at N=512). Band rides as the
    moving operand, image data as the stationary one, fusing conv+transpose.
    Strided column selection keeps both stages on the single p-major band:
        stage1[m, n] = sum_r s[r, 4m+cb] * band(r, n)    -> t partitions are w=4m+cb
        stage2[m, n] = sum_w t[w, 4m+hb] * band(w, n)    -> final rows h=4m+hb
  - Charbonnier on ACT: Square (PSUM->SBUF), Sqrt(x + eps) with accum_out
    collecting per-partition sums into acc[128, 8]; acc is DMA'd out and
    the host reduces it (with the cross-core sum) in float64.
"""

import numpy as np

import concourse.bass as bass
import concourse.bacc as bacc
import concourse.mybir as mybir
import concourse.tile as tile
from concourse.bass_interp import get_hw_module
from concourse.bass_utils import run_bass_kernel_spmd

N_CORES = 8
B_TOTAL = 16
B_PER_CORE = B_TOTAL // N_CORES
CH = 3
H = W = 512
P = 128
NCHUNK = H // P  # 4
EPS = 1e-6
F32 = mybir.dt.float32
F32R = mybir.dt.float32r
AF = mybir.ActivationFunctionType


def make_band() -> np.ndarray:
    """[128, 4, 512] p-major band: band[p, slot, n] = 1/7 if |4p+slot-n| <= 3."""
    band = np.zeros((P, NCHUNK, W), dtype=np.float32)
    p = np.arange(P)[:, None, None]
    slot = np.arange(NCHUNK)[None, :, None]
    n = np.arange(W)[None, None, :]
    band[np.abs(4 * p + slot - n) <= 3] = np.float32(1.0) / np.float32(7.0)
    return band


def build_program() -> tuple[bacc.Bacc, str, str, str, str]:
    nc = bacc.Bacc("TRN2", target_bir_lowering=False, debug=False, num_devices=N_CORES)

    x = nc.dram_tensor("x", [B_PER_CORE, CH, H, W], F32, kind="ExternalInput")
    y = nc.dram_tensor("y", [B_PER_CORE, CH, H, W], F32, kind="ExternalInput")
    band = nc.dram_tensor("band", [P, NCHUNK, W], F32R, kind="ExternalInput")
    out = nc.dram_tensor("out", [P, B_PER_CORE * NCHUNK], F32, kind="ExternalOutput")

    with tile.TileContext(nc) as tc:
        with (
            tc.tile_pool(name="const", bufs=1) as cpool,
            tc.tile_pool(name="xy", bufs=1) as xypool,
            tc.tile_pool(name="data", bufs=2) as dpool,
            tc.tile_pool(name="small", bufs=2) as spool,
            tc.tile_pool(name="psum", bufs=2, space="PSUM") as ppool,
        ):
            # band halves ride at the head of each HWDGE ring
            band_t = cpool.tile([P, NCHUNK, W], F32R)
            nc.sync.dma_start(band_t[:, 0:2, :], band.ap()[:, 0:2, :])
            nc.scalar.dma_start(band_t[:, 2:4, :], band.ap()[:, 2:4, :])

            epsb = cpool.tile([P, 1], F32)
            nc.gpsimd.memset(epsb[:], float(EPS))

            acc = cpool.tile([P, B_PER_CORE * NCHUNK], F32)

            # per-channel 1MB pieces: x on the SP ring, y on the ACT ring,
            # issued image-by-image so pieces pair up in time.
            xt, yt = [], []
            for b in range(B_PER_CORE):
                xb = xypool.tile([P, CH, NCHUNK, W], F32, tag=f"x{b}")
                yb = xypool.tile([P, CH, NCHUNK, W], F32, tag=f"y{b}")
                for ch in range(CH):
                    nc.sync.dma_start(
                        xb[:, ch, :, :],
                        x.ap()[b, ch].rearrange("(p c) w -> p c w", c=NCHUNK),
                    )
                    nc.scalar.dma_start(
                        yb[:, ch, :, :],
                        y.ap()[b, ch].rearrange("(p c) w -> p c w", c=NCHUNK),
                    )
                xt.append(xb)
                yt.append(yb)

            for b in range(B_PER_CORE):
                xb, yb = xt[b], yt[b]
                # s = sum_ch (x - y); partial sums as channels arrive
                d = xypool.tile([P, CH, NCHUNK, W], F32, tag="d")
                e = dpool.tile([P, NCHUNK, W], F32, tag="e")
                s = dpool.tile([P, NCHUNK, W // 4, 4], F32R, tag="s")
                sv = s.rearrange("p c w4 f -> p c (w4 f)")
                for ch in range(CH):
                    nc.vector.tensor_sub(d[:, ch, :, :], xb[:, ch, :, :], yb[:, ch, :, :])
                nc.vector.tensor_add(e[:], d[:, 0, :, :], d[:, 1, :, :])
                nc.vector.tensor_add(sv[:], e[:], d[:, 2, :, :])

                # stage 1: vertical conv + transpose; column-select w = 4m+cb
                t = dpool.tile([P, NCHUNK, W // 4, 4], F32R, tag="t")
                for cb in range(NCHUNK):
                    ps1 = ppool.tile([P, W], F32, tag="ps1")
                    for c in range(NCHUNK):
                        nc.tensor.matmul(
                            ps1[:],
                            s[:, c, :, cb],
                            band_t[:, c, :],
                            start=(c == 0),
                            stop=(c == NCHUNK - 1),
                        )
                    nc.scalar.copy(
                        t[:, cb, :, :].rearrange("p w4 f -> p (w4 f)"), ps1[:]
                    )

                # stage 2: horizontal conv, rows back as h = 4m+hb
                for hb in range(NCHUNK):
                    ps2 = ppool.tile([P, W], F32, tag="ps2")
                    for cb in range(NCHUNK):
                        nc.tensor.matmul(
                            ps2[:],
                            t[:, cb, :, hb],
                            band_t[:, cb, :],
                            start=(cb == 0),
                            stop=(cb == NCHUNK - 1),
                        )
                    sq = spool.tile([P, W], F32, tag="sq")
                    nc.scalar.activation(sq[:], ps2[:], AF.Square)
                    u = spool.tile([P, W], F32, tag="u")
                    col = b * NCHUNK + hb
                    nc.scalar.activation(
                        u[:], sq[:], AF.Sqrt, bias=epsb[:],
                        accum_out=acc[:, col:col + 1],
                    )

            nc.sync.dma_start(out.ap()[:], acc[:])

    nc.compile()
    nc.m = get_hw_module(nc.m)
    return nc, x.name, y.name, band.name, out.name


_CACHE = {}


def _get_program():
    if "prog" not in _CACHE:
        _CACHE["prog"] = build_program()
    return _CACHE["prog"]


def run_sharded(x: np.ndarray, y: np.ndarray, trace: bool = False):
    """Run the SPMD kernel; returns (per-core sums list, BassKernelResults)."""
    nc, xname, yname, bandname, outname = _get_program()
    band = make_band()
    x = np.ascontiguousarray(np.asarray(x, dtype=np.float32))
    y = np.ascontiguousarray(np.asarray(y, dtype=np.float32))
    in_maps = []
    for k in range(N_CORES):
        sl = slice(k * B_PER_CORE, (k + 1) * B_PER_CORE)
        in_maps.append({
            xname: x[sl],
            yname: y[sl],
            bandname: band,
        })
    res = run_bass_kernel_spmd(
        nc, in_maps, core_ids=list(range(N_CORES)), trace=trace
    )
    sums = [float(res.results[k][outname].astype(np.float64).sum())
            for k in range(N_CORES)]
    return sums, res


def kernel(x: np.ndarray, y: np.ndarray) -> np.ndarray:
    sums, _ = run_sharded(x, y)
    total = float(np.sum(np.asarray(sums, dtype=np.float64)))
    return np.float32(total / (B_TOTAL * H * W))
